# Initial kernel scaffold
#
"""Two-layer GAT on 8 Trainium2 NeuronCores (Bass/Tile).

Strategy (per core c of 8):
  - nodes sharded: core c owns rows [c*12500, (c+1)*12500); edges partitioned
    by destination core (host-side, static index prep).
  - GEMM1 computes [h1 | s1src | s1dst | skip1] in one pass from x-shard
    (scores folded via W1@a; biases folded via a ones-row in padded x).
  - AllGather of [h1 | s1src | 1 | pad] rows (bf16) -> full gather table.
  - Edge phase: per 128-node tile, per 128-edge chunk: indirect-DMA gather of
    source rows, p = exp(leakyrelu(ssrc+sdst)) (= max(exp(u), exp(0.2u))),
    scaled one-hot lhsT built by one fused DVE op, matmul-accumulate into
    PSUM [128 nodes, 132]; col 129 accumulates the softmax denominator
    (gathered rows carry a constant 1.0 there).
  - Layer-2 aggregation runs in 128-dim h1relu space (aggregation commutes
    with @W2), so layer 2 also gathers 264-byte rows; out = (agg2/denom)@W2
    + h1relu@lin2_W + biases.
"""
import sys

if '/opt/trn_rl_repo' not in sys.path:
    sys.path.insert(0, '/opt/trn_rl_repo')

import numpy as np

P = 128
NEG = 0.2
AGW = 132          # gathered row width: [feat(128) | ssrc | one | pad pad]
EPS = 1e-16


def _cfg(N, E, DIN, D1, D2, ncore, group_tiles):
    assert N % ncore == 0
    nloc = N // ncore
    nt = -(-nloc // P)
    cfg = dict(N=N, E=E, DIN=DIN, D1=D1, D2=D2, NCORE=ncore,
               NLOC=nloc, NT=nt, NLOCP=nt * P,
               K1=-(-(DIN + 1) // P),  # input dim padded to K1*128 (incl ones row)
               GT=group_tiles)
    return cfg


def preprocess(x, edge_index, cfg, rng_pad_src=0):
    """Host-side index prep. Returns per-core aux arrays + cpt."""
    N, ncore, nloc, nt = cfg['N'], cfg['NCORE'], cfg['NLOC'], cfg['NT']
    nlocp = cfg['NLOCP']
    src, dst = np.asarray(edge_index[0]), np.asarray(edge_index[1])

    core_of = dst // nloc
    per_core = []
    cpt = 1
    for c in range(ncore):
        m = core_of == c
        s, d = src[m], dst[m] - c * nloc
        t = d // P
        counts = np.bincount(t, minlength=nt)
        cpt = max(cpt, int(-(-counts.max() // P)))
        per_core.append((s, d, t))

    out = []
    for c in range(ncore):
        s, d, t = per_core[c]
        order = np.argsort(t, kind='stable')
        s, d, t = s[order], d[order], t[order]
        starts = np.zeros(nt + 1, np.int64)
        np.cumsum(np.bincount(t, minlength=nt), out=starts[1:])

        nch = nt * cpt
        src_gidx = np.zeros((nt, cpt * P), np.int32)
        sdst_idx = np.zeros((nt, cpt * P), np.int32)
        dstloc_f = np.full((nt, cpt * P), -1.0, np.float32)
        for ti in range(nt):
            e0, e1 = starts[ti], starts[ti + 1]
            n = e1 - e0
            sv = s[e0:e1]
            dv = d[e0:e1]
            src_gidx[ti, :n] = (sv // nloc) * nlocp + (sv % nloc)
            pv = dv % P
            sdst_idx[ti, :n] = pv * nt + ti
            dstloc_f[ti, :n] = pv
        # chunk (ti, j) slots -> SBUF layout [128, nch]: col q = ti*cpt + j
        def to_sb(a):
            return np.ascontiguousarray(
                a.reshape(nt, cpt, P).transpose(2, 0, 1).reshape(P, nch))
        # wait: need [P, nt*cpt] with col q=(ti*cpt+j) holding slot axis P
        out.append(dict(
            src_gidx=to_sb(src_gidx),
            sdst_idx=to_sb(sdst_idx),
            dstloc_f=to_sb(dstloc_f),
        ))
    return out, cpt


def make_xt_tiles(x, cfg, c):
    """Per-core transposed, padded, tiled x: [NT*K1*128, 128] fp32."""
    nloc, nt, k1, din = cfg['NLOC'], cfg['NT'], cfg['K1'], cfg['DIN']
    nlocp = nt * P
    xl = np.zeros((nlocp, k1 * P), np.float32)
    xl[:nloc, :din] = x[c * nloc:(c + 1) * nloc]
    xl[:nloc, din] = 1.0  # ones column for folded biases
    blocks = np.zeros((nt, k1, P, P), np.float32)
    for t in range(nt):
        blk = xl[t * P:(t + 1) * P]               # [128 nodes, K1*128 feats]
        blocks[t] = blk.reshape(P, k1, P).transpose(1, 2, 0)  # [k][feat][node]
    return np.ascontiguousarray(blocks.reshape(nt * k1 * P, P))


def build(cfg, cpt, weights):
    """Build the Bass program. weights: dict of numpy arrays (fp32)."""
    import concourse.bass as bass
    import concourse.bacc as bacc
    import concourse.mybir as mybir
    import concourse.tile as tile

    N, DIN, D1, D2 = cfg['N'], cfg['DIN'], cfg['D1'], cfg['D2']
    ncore, nloc, nt, nlocp, k1 = (cfg['NCORE'], cfg['NLOC'], cfg['NT'],
                                  cfg['NLOCP'], cfg['K1'])
    gt = cfg['GT']
    nch = nt * cpt
    ngrp = -(-nt // gt)
    assert D1 == P

    W1, a_src1, a_dst1, b1 = (weights[k] for k in ('W1', 'a_src1', 'a_dst1', 'b1'))
    lin1_W, lin1_b = weights['lin1_W'], weights['lin1_b']
    W2, a_src2, a_dst2, b2 = (weights[k] for k in ('W2', 'a_src2', 'a_dst2', 'b2'))
    lin2_W, lin2_b = weights['lin2_W'], weights['lin2_b']

    # ---- host-derived constants -------------------------------------------
    c1 = 2 * D1 + 2
    w1aug = np.zeros((k1 * P, c1), np.float32)
    w1aug[:DIN, 0:D1] = W1
    w1aug[:DIN, D1] = W1 @ a_src1
    w1aug[:DIN, D1 + 1] = W1 @ a_dst1
    w1aug[:DIN, D1 + 2:] = lin1_W
    w1aug[DIN, D1 + 2:] = b1 + lin1_b          # ones-row -> biases into skip
    w2a = np.stack([W2 @ a_src2, W2 @ a_dst2], axis=1).astype(np.float32)
    bias2 = np.tile((b2 + lin2_b)[None, :], (P, 1)).astype(np.float32)
    iota_np = np.tile(np.arange(P, dtype=np.float32), (P, 1))
    ident_np = np.eye(P, dtype=np.float32)

    nc = bacc.Bacc("TRN2", target_bir_lowering=False, debug=False,
                   num_devices=ncore)
    f32, bf16, i32 = mybir.dt.float32, mybir.dt.bfloat16, mybir.dt.int32

    # ---- I/O ---------------------------------------------------------------
    xt_t = nc.dram_tensor("xt_tiles", [nt * k1 * P, P], f32, kind="ExternalInput")
    idx_t = nc.dram_tensor("src_gidx", [P, nch], i32, kind="ExternalInput")
    sidx_t = nc.dram_tensor("sdst_idx", [P, nch], i32, kind="ExternalInput")
    dstf_t = nc.dram_tensor("dstloc_f", [P, nch], f32, kind="ExternalInput")
    out_t = nc.dram_tensor("out", [nloc, D2], f32, kind="ExternalOutput")

    w1aug_c = nc.inline_tensor(w1aug, name="w1aug")
    w2a_c = nc.inline_tensor(w2a, name="w2a")
    w2_c = nc.inline_tensor(W2.astype(np.float32), name="w2c")
    lin2_c = nc.inline_tensor(lin2_W.astype(np.float32), name="lin2c")
    bias2_c = nc.inline_tensor(bias2, name="bias2c")
    iota_c = nc.inline_tensor(iota_np, name="iotac")
    ident_c = nc.inline_tensor(ident_np, name="identc")

    rg = [list(range(ncore))]

    with tile.TileContext(nc) as tc:
        with (
            tc.tile_pool(name="persist", bufs=1) as pp,
            tc.tile_pool(name="work", bufs=2) as wp,
            tc.tile_pool(name="moh", bufs=4) as mp,
            tc.tile_pool(name="gath", bufs=3) as gp,
            tc.tile_pool(name="psum", bufs=2, space="PSUM") as pep,
            tc.tile_pool(name="dram", bufs=1, space="DRAM") as dp,
        ):
            # ---- persistent tiles -----------------------------------------
            iota_sb = pp.tile([P, P], f32)
            nc.sync.dma_start(out=iota_sb[:], in_=iota_c.ap())
            ident_sb = pp.tile([P, P], f32)
            nc.sync.dma_start(out=ident_sb[:], in_=ident_c.ap())
            w1_sb = pp.tile([P, k1, c1], f32)
            nc.sync.dma_start(
                out=w1_sb[:],
                in_=w1aug_c.ap().rearrange("(k r) c -> r k c", k=k1))
            w2a_sb = pp.tile([P, 2], f32)
            nc.sync.dma_start(out=w2a_sb[:], in_=w2a_c.ap())
            w2_sb = pp.tile([P, D2], f32)
            nc.sync.dma_start(out=w2_sb[:], in_=w2_c.ap())
            lin2_sb = pp.tile([P, D2], f32)
            nc.sync.dma_start(out=lin2_sb[:], in_=lin2_c.ap())
            bias2_sb = pp.tile([P, D2], f32)
            nc.sync.dma_start(out=bias2_sb[:], in_=bias2_c.ap())
            idx_sb = pp.tile([P, nch], i32)
            nc.sync.dma_start(out=idx_sb[:], in_=idx_t[:, :])
            sidx_sb = pp.tile([P, nch], i32)
            nc.sync.dma_start(out=sidx_sb[:], in_=sidx_t[:, :])
            dstf_sb = pp.tile([P, nch], f32)
            nc.sync.dma_start(out=dstf_sb[:], in_=dstf_t[:, :])

            h1relu_sb = pp.tile([P, nt * P], f32)
            skip1_sb = pp.tile([P, nt * P], f32)
            sdst1_sb = pp.tile([P, nt], f32)
            sdst2_sb = pp.tile([P, nt], f32)
            esd1_sb = pp.tile([P, nch], f32)
            esd2_sb = pp.tile([P, nch], f32)

            # ---- DRAM buffers ---------------------------------------------
            ag1_in = dp.tile([nlocp, AGW], bf16)
            ag1_out = dp.tile([ncore * nlocp, AGW], bf16, addr_space="Shared")
            ag2_in = dp.tile([nlocp, AGW], bf16)
            ag2_out = dp.tile([ncore * nlocp, AGW], bf16, addr_space="Shared")
            sdst1_dram = dp.tile([P * nt, 1], f32)
            sdst2_dram = dp.tile([P * nt, 1], f32)

            # ================= Phase 1: GEMM1 ==============================
            for t in range(nt):
                xt_sb = wp.tile([P, k1, P], f32, tag="xt", bufs=3)
                nc.sync.dma_start(
                    out=xt_sb[:],
                    in_=xt_t[t * k1 * P:(t + 1) * k1 * P, :]
                        .rearrange("(k r) c -> r k c", k=k1))
                ps1 = pep.tile([P, c1], f32, tag="g1", space="PSUM")
                for k in range(k1):
                    nc.tensor.matmul(out=ps1[:], lhsT=xt_sb[:, k, :],
                                     rhs=w1_sb[:, k, :],
                                     start=(k == 0), stop=(k == k1 - 1))
                agt = wp.tile([P, AGW], bf16, tag="ag1t", bufs=3)
                nc.vector.tensor_copy(out=agt[:, 0:D1 + 1], in_=ps1[:, 0:D1 + 1])
                nc.vector.memset(agt[:, D1 + 1:D1 + 2], 1.0)
                nc.vector.memset(agt[:, D1 + 2:AGW], 0.0)
                nc.sync.dma_start(out=ag1_in[t * P:(t + 1) * P, :], in_=agt[:])
                nc.vector.tensor_copy(out=sdst1_sb[:, t:t + 1],
                                      in_=ps1[:, D1 + 1:D1 + 2])
                nc.scalar.copy(out=skip1_sb[:, t * P:(t + 1) * P],
                               in_=ps1[:, D1 + 2:c1])
            nc.sync.dma_start(
                out=sdst1_dram[:].rearrange("(p t) e -> p (t e)", p=P),
                in_=sdst1_sb[:])
            nc.gpsimd.collective_compute(
                "AllGather", mybir.AluOpType.bypass, replica_groups=rg,
                ins=[ag1_in[:].opt()], outs=[ag1_out[:].opt()])

            # ============ Edge phase (shared for both layers) ==============
            def edge_phase(ag_out, sdst_dram, esd_sb, evict_tile):
                for g in range(ngrp):
                    t0, t1 = g * gt, min((g + 1) * gt, nt)
                    q0, q1 = t0 * cpt, t1 * cpt
                    gc = q1 - q0
                    nc.gpsimd.indirect_dma_start(
                        out=esd_sb[:, q0:q1].rearrange("p (k e) -> p k e", e=1),
                        out_offset=None,
                        in_=sdst_dram[:],
                        in_offset=bass.IndirectOffsetOnAxis(
                            ap=sidx_sb[:, q0:q1], axis=0))
                    g_sb = gp.tile([P, gc * AGW], bf16, tag="g")
                    nc.gpsimd.indirect_dma_start(
                        out=g_sb[:].rearrange("p (k e) -> p k e", k=gc),
                        out_offset=None,
                        in_=ag_out[:],
                        in_offset=bass.IndirectOffsetOnAxis(
                            ap=idx_sb[:, q0:q1], axis=0))
                    u = wp.tile([P, gc], f32, tag="u")
                    nc.vector.tensor_tensor(
                        out=u[:], in0=esd_sb[:, q0:q1],
                        in1=g_sb[:].rearrange("p (k e) -> p k e", k=gc)[:, :, D1],
                        op=mybir.AluOpType.add)
                    e1 = wp.tile([P, gc], f32, tag="e1")
                    nc.scalar.activation(out=e1[:], in_=u[:],
                                         func=mybir.ActivationFunctionType.Exp)
                    e2 = wp.tile([P, gc], f32, tag="e2")
                    nc.scalar.activation(out=e2[:], in_=u[:],
                                         func=mybir.ActivationFunctionType.Exp,
                                         scale=NEG)
                    pt = wp.tile([P, gc], f32, tag="p")
                    nc.vector.tensor_tensor(out=pt[:], in0=e1[:], in1=e2[:],
                                            op=mybir.AluOpType.max)
                    for t in range(t0, t1):
                        pse = pep.tile([P, AGW], f32, tag="edge", space="PSUM")
                        for j in range(cpt):
                            q = t * cpt + j
                            lj = q - q0
                            m_oh = mp.tile([P, P], bf16, tag="moh")
                            nc.vector.tensor_scalar(
                                out=m_oh[:], in0=iota_sb[:],
                                scalar1=dstf_sb[:, q:q + 1],
                                scalar2=pt[:, lj:lj + 1],
                                op0=mybir.AluOpType.is_equal,
                                op1=mybir.AluOpType.mult)
                            nc.tensor.matmul(
                                out=pse[:], lhsT=m_oh[:],
                                rhs=g_sb[:, lj * AGW:(lj + 1) * AGW],
                                start=(j == 0), stop=(j == cpt - 1))
                        evict_tile(t, pse)

            # ---- layer-1 eviction: h1relu + GEMM2a + ag2 assembly ---------
            def evict1(t, pse):
                den = wp.tile([P, 1], f32, tag="den")
                nc.vector.tensor_scalar(out=den[:], in0=pse[:, D1 + 1:D1 + 2],
                                        scalar1=EPS, scalar2=None,
                                        op0=mybir.AluOpType.add)
                rec = wp.tile([P, 1], f32, tag="rec")
                nc.vector.reciprocal(out=rec[:], in_=den[:])
                tmp = wp.tile([P, P], f32, tag="ev1a")
                nc.vector.tensor_scalar(out=tmp[:], in0=pse[:, 0:D1],
                                        scalar1=rec[:, 0:1], scalar2=None,
                                        op0=mybir.AluOpType.mult)
                tmp2 = wp.tile([P, P], f32, tag="ev1b")
                nc.vector.tensor_tensor(out=tmp2[:], in0=tmp[:],
                                        in1=skip1_sb[:, t * P:(t + 1) * P],
                                        op=mybir.AluOpType.add)
                nc.scalar.activation(out=h1relu_sb[:, t * P:(t + 1) * P],
                                     in_=tmp2[:],
                                     func=mybir.ActivationFunctionType.Relu)
                # GEMM2a: transpose h1relu tile, compute [s2src, s2dst]
                tp = pep.tile([P, P], f32, tag="t", space="PSUM")
                nc.tensor.transpose(out=tp[:],
                                    in_=h1relu_sb[:, t * P:(t + 1) * P],
                                    identity=ident_sb[:])
                h1T = wp.tile([P, P], f32, tag="h1T")
                nc.scalar.copy(out=h1T[:], in_=tp[:])
                sc = pep.tile([P, 2], f32, tag="g1", space="PSUM")
                nc.tensor.matmul(out=sc[:], lhsT=h1T[:], rhs=w2a_sb[:],
                                 start=True, stop=True)
                agt = wp.tile([P, AGW], bf16, tag="ag2t", bufs=3)
                nc.vector.tensor_copy(out=agt[:, 0:D1],
                                      in_=h1relu_sb[:, t * P:(t + 1) * P])
                nc.vector.tensor_copy(out=agt[:, D1:D1 + 1], in_=sc[:, 0:1])
                nc.vector.memset(agt[:, D1 + 1:D1 + 2], 1.0)
                nc.vector.memset(agt[:, D1 + 2:AGW], 0.0)
                nc.sync.dma_start(out=ag2_in[t * P:(t + 1) * P, :], in_=agt[:])
                nc.vector.tensor_copy(out=sdst2_sb[:, t:t + 1], in_=sc[:, 1:2])

            edge_phase(ag1_out, sdst1_dram, esd1_sb, evict1)
            nc.sync.dma_start(
                out=sdst2_dram[:].rearrange("(p t) e -> p (t e)", p=P),
                in_=sdst2_sb[:])
            nc.gpsimd.collective_compute(
                "AllGather", mybir.AluOpType.bypass, replica_groups=rg,
                ins=[ag2_in[:].opt()], outs=[ag2_out[:].opt()])

            # ---- layer-2 eviction: out = (agg2/den)@W2 + h1relu@lin2 + b --
            def evict2(t, pse):
                den = wp.tile([P, 1], f32, tag="den")
                nc.vector.tensor_scalar(out=den[:], in0=pse[:, D1 + 1:D1 + 2],
                                        scalar1=EPS, scalar2=None,
                                        op0=mybir.AluOpType.add)
                rec = wp.tile([P, 1], f32, tag="rec")
                nc.vector.reciprocal(out=rec[:], in_=den[:])
                a2n = wp.tile([P, P], f32, tag="a2n")
                nc.vector.tensor_scalar(out=a2n[:], in0=pse[:, 0:D1],
                                        scalar1=rec[:, 0:1], scalar2=None,
                                        op0=mybir.AluOpType.mult)
                tp1 = pep.tile([P, P], f32, tag="t", space="PSUM")
                nc.tensor.transpose(out=tp1[:], in_=a2n[:], identity=ident_sb[:])
                a2T = wp.tile([P, P], f32, tag="a2T")
                nc.scalar.copy(out=a2T[:], in_=tp1[:])
                tp2 = pep.tile([P, P], f32, tag="t", space="PSUM")
                nc.tensor.transpose(out=tp2[:],
                                    in_=h1relu_sb[:, t * P:(t + 1) * P],
                                    identity=ident_sb[:])
                h1T = wp.tile([P, P], f32, tag="h1T")
                nc.scalar.copy(out=h1T[:], in_=tp2[:])
                po = pep.tile([P, D2], f32, tag="out", space="PSUM")
                nc.tensor.matmul(out=po[:], lhsT=a2T[:], rhs=w2_sb[:],
                                 start=True, stop=False)
                nc.tensor.matmul(out=po[:], lhsT=h1T[:], rhs=lin2_sb[:],
                                 start=False, stop=True)
                osb = wp.tile([P, D2], f32, tag="osb", bufs=3)
                nc.vector.tensor_tensor(out=osb[:], in0=po[:], in1=bias2_sb[:],
                                        op=mybir.AluOpType.add)
                rows = min(nloc - t * P, P)
                nc.sync.dma_start(out=out_t[t * P:t * P + rows, :],
                                  in_=osb[:rows, :])

            edge_phase(ag2_out, sdst2_dram, esd2_sb, evict2)

    nc.compile()
    return nc


def _run(nc, in_maps, ncore, trace=False):
    from concourse import bass_utils
    return bass_utils.run_bass_kernel_spmd(
        nc, in_maps, core_ids=list(range(ncore)), trace=trace)


_CACHE = {}


def kernel(**inputs):
    x = np.asarray(inputs['x'], np.float32)
    edge_index = np.asarray(inputs['edge_index'], np.int32)
    cfg = _cfg(N=100000, E=400000, DIN=300, D1=128, D2=512, ncore=8,
               group_tiles=7)
    weights = {k: np.asarray(v, np.float32) for k, v in inputs.items()
               if k not in ('x', 'edge_index')}

    aux, cpt = preprocess(x, edge_index, cfg)
    key = ('nn_gat', cpt)
    if key not in _CACHE:
        _CACHE[key] = build(cfg, cpt, weights)
    nc = _CACHE[key]

    in_maps = []
    for c in range(cfg['NCORE']):
        in_maps.append(dict(
            xt_tiles=make_xt_tiles(x, cfg, c),
            src_gidx=aux[c]['src_gidx'],
            sdst_idx=aux[c]['sdst_idx'],
            dstloc_f=aux[c]['dstloc_f'],
        ))
    res = _run(nc, in_maps, cfg['NCORE'])
    out = np.concatenate([res.results[c]['out'] for c in range(cfg['NCORE'])],
                         axis=0)
    return out


# revision 3
# speedup vs baseline: 1.0057x; 1.0057x over previous
"""Two-layer GAT on 8 Trainium2 NeuronCores (Bass/Tile).

Sharding: nodes split 12500/core; edges partitioned by destination core.
Per core, per layer:
  GEMM phase computes [h | s_src | s_dst | skip] columns in one pass
  (attention vectors folded as W@a columns; biases folded via a ones-row).
  A bf16 [h | s_src | 1 | pad] table is AllGathered (132 cols, 264B rows).
  Edge phase: per 128-edge chunk (chunks grouped per 128-node dst tile):
    - indirect-DMA gather of the chunk's source rows -> [128, 132] bf16
    - e_sdst expansion via tensor_tensor_reduce against a per-tile
      broadcast of local s_dst (PE-transpose trick)
    - p = exp(leaky_relu(ssrc+sdst)) computed as max(exp(u), exp(0.2u))
    - scaled one-hot lhsT: OH01 = (iota == dstloc) on DVE, m_oh = OH01 * p
      on ACT; matmul-accumulate psum[128 nodes, 132]; column 129
      accumulates the softmax denominator (rows carry a constant 1.0).
  Layer 2 aggregates in 128-dim h1relu space (aggregation commutes with
  @W2): out = (agg2/denom)@W2 + h1relu@lin2_W + biases.
"""
import sys

if '/opt/trn_rl_repo' not in sys.path:
    sys.path.insert(0, '/opt/trn_rl_repo')

import numpy as np

P = 128
NEG = 0.2
AGW = 132          # gathered row: [feat(128) | ssrc | one | pad pad]
EPS = 1e-16


def _cfg(N, E, DIN, D1, D2, ncore, group_chunks=36):
    assert N % ncore == 0
    nloc = N // ncore
    nt = -(-nloc // P)
    return dict(N=N, E=E, DIN=DIN, D1=D1, D2=D2, NCORE=ncore,
                NLOC=nloc, NT=nt, NLOCP=nt * P,
                K1=-(-(DIN + 1) // P), GC=group_chunks)


def preprocess(x, edge_index, cfg):
    """Host-side index prep.

    Returns (aux_per_core, cpt_t) where cpt_t[t] = chunks for tile t
    (max over cores, so the SPMD program structure is shared).
    aux arrays are laid out [128, nch_total] in global chunk order.
    """
    ncore, nloc, nt = cfg['NCORE'], cfg['NLOC'], cfg['NT']
    nlocp = cfg['NLOCP']
    src = np.asarray(edge_index[0], np.int64)
    dst = np.asarray(edge_index[1], np.int64)

    per_core = []
    counts_all = np.zeros((ncore, nt), np.int64)
    for c in range(ncore):
        m = (dst // nloc) == c
        s, d = src[m], dst[m] - c * nloc
        t = d // P
        order = np.argsort(t, kind='stable')
        s, d, t = s[order], d[order], t[order]
        counts = np.bincount(t, minlength=nt)
        counts_all[c] = counts
        starts = np.zeros(nt + 1, np.int64)
        np.cumsum(counts, out=starts[1:])
        per_core.append((s, d, starts))

    cpt_t = np.maximum(1, -(-counts_all.max(axis=0) // P))  # [nt]
    nch = int(cpt_t.sum())
    chunk_t0 = np.zeros(nt + 1, np.int64)
    np.cumsum(cpt_t, out=chunk_t0[1:])

    aux = []
    for c in range(ncore):
        s, d, starts = per_core[c]
        src_gidx = np.zeros((nch, P), np.int32)
        dstloc_f = np.full((nch, P), -1.0, np.float32)
        for t in range(nt):
            e0, e1 = starts[t], starts[t + 1]
            n = e1 - e0
            q0 = chunk_t0[t]
            sv = s[e0:e1]
            buf_s = np.zeros(int(cpt_t[t]) * P, np.int32)
            buf_d = np.full(int(cpt_t[t]) * P, -1.0, np.float32)
            buf_s[:n] = (sv // nloc) * nlocp + (sv % nloc)
            buf_d[:n] = (d[e0:e1] % P).astype(np.float32)
            src_gidx[q0:q0 + cpt_t[t]] = buf_s.reshape(-1, P)
            dstloc_f[q0:q0 + cpt_t[t]] = buf_d.reshape(-1, P)
        aux.append(dict(src_gidx=np.ascontiguousarray(src_gidx.T),
                        dstloc_f=np.ascontiguousarray(dstloc_f.T)))
    return aux, cpt_t


def make_xt_tiles(x, cfg, c):
    """Per-core transposed, padded, tiled x: [NT*K1*128, 128] fp32."""
    nloc, nt, k1, din = cfg['NLOC'], cfg['NT'], cfg['K1'], cfg['DIN']
    nlocp = nt * P
    xl = np.zeros((nlocp, k1 * P), np.float32)
    xl[:nloc, :din] = x[c * nloc:(c + 1) * nloc]
    xl[:nloc, din] = 1.0  # ones column feeds folded biases
    blocks = np.zeros((nt, k1, P, P), np.float32)
    for t in range(nt):
        blk = xl[t * P:(t + 1) * P]
        blocks[t] = blk.reshape(P, k1, P).transpose(1, 2, 0)
    return np.ascontiguousarray(blocks.reshape(nt * k1 * P, P))


def build(cfg, cpt_t, weights):
    import concourse.bass as bass
    import concourse.bacc as bacc
    import concourse.mybir as mybir
    import concourse.tile as tile

    DIN, D1, D2 = cfg['DIN'], cfg['D1'], cfg['D2']
    ncore, nloc, nt, nlocp, k1 = (cfg['NCORE'], cfg['NLOC'], cfg['NT'],
                                  cfg['NLOCP'], cfg['K1'])
    gc_target = cfg['GC']
    nch = int(cpt_t.sum())
    chunk_t0 = np.zeros(nt + 1, np.int64)
    np.cumsum(cpt_t, out=chunk_t0[1:])
    assert D1 == P

    # group tiles so each group has ~gc_target chunks
    groups = []
    t0 = 0
    while t0 < nt:
        t1 = t0 + 1
        while t1 < nt and chunk_t0[t1 + 1] - chunk_t0[t0] <= gc_target:
            t1 += 1
        groups.append((t0, t1))
        t0 = t1

    W1, a_src1, a_dst1, b1 = (weights[k] for k in ('W1', 'a_src1', 'a_dst1', 'b1'))
    lin1_W, lin1_b = weights['lin1_W'], weights['lin1_b']
    W2, a_src2, a_dst2, b2 = (weights[k] for k in ('W2', 'a_src2', 'a_dst2', 'b2'))
    lin2_W, lin2_b = weights['lin2_W'], weights['lin2_b']

    c1 = 2 * D1 + 2
    w1aug = np.zeros((k1 * P, c1), np.float32)
    w1aug[:DIN, 0:D1] = W1
    w1aug[:DIN, D1] = W1 @ a_src1
    w1aug[:DIN, D1 + 1] = W1 @ a_dst1
    w1aug[:DIN, D1 + 2:] = lin1_W
    w1aug[DIN, D1 + 2:] = b1 + lin1_b
    w2a = np.stack([W2 @ a_src2, W2 @ a_dst2], axis=1).astype(np.float32)
    bias2 = np.tile((b2 + lin2_b)[None, :], (P, 1)).astype(np.float32)
    iota_np = np.tile(np.arange(P, dtype=np.float32), (P, 1))
    ident_np = np.eye(P, dtype=np.float32)

    nc = bacc.Bacc("TRN2", target_bir_lowering=False, debug=False,
                   num_devices=ncore)
    f32, bf16, i32 = mybir.dt.float32, mybir.dt.bfloat16, mybir.dt.int32

    xt_t = nc.dram_tensor("xt_tiles", [nt * k1 * P, P], f32, kind="ExternalInput")
    idx_t = nc.dram_tensor("src_gidx", [P, nch], i32, kind="ExternalInput")
    dstf_t = nc.dram_tensor("dstloc_f", [P, nch], f32, kind="ExternalInput")
    out_t = nc.dram_tensor("out", [nloc, D2], f32, kind="ExternalOutput")

    w1aug_c = nc.inline_tensor(w1aug, name="w1aug")
    w2a_c = nc.inline_tensor(w2a, name="w2a")
    w2_c = nc.inline_tensor(W2.astype(np.float32), name="w2c")
    lin2_c = nc.inline_tensor(lin2_W.astype(np.float32), name="lin2c")
    bias2_c = nc.inline_tensor(bias2, name="bias2c")
    iota_c = nc.inline_tensor(iota_np, name="iotac")
    ident_c = nc.inline_tensor(ident_np, name="identc")

    rg = [list(range(ncore))]

    with tile.TileContext(nc) as tc:
        with (
            tc.tile_pool(name="persist", bufs=1) as pp,
            tc.tile_pool(name="work", bufs=2) as wp,
            tc.tile_pool(name="moh", bufs=6) as mp,
            tc.tile_pool(name="gath", bufs=3) as gp,
            tc.tile_pool(name="psum", bufs=2, space="PSUM") as pep,
            tc.tile_pool(name="dram", bufs=1, space="DRAM") as dp,
        ):
            iota_sb = pp.tile([P, P], f32)
            nc.sync.dma_start(out=iota_sb[:], in_=iota_c.ap())
            ident_sb = pp.tile([P, P], f32)
            nc.sync.dma_start(out=ident_sb[:], in_=ident_c.ap())
            w1_sb = pp.tile([P, k1, c1], f32)
            nc.sync.dma_start(
                out=w1_sb[:],
                in_=w1aug_c.ap().rearrange("(k r) c -> r k c", k=k1))
            w2a_sb = pp.tile([P, 2], f32)
            nc.sync.dma_start(out=w2a_sb[:], in_=w2a_c.ap())
            w2_sb = pp.tile([P, D2], f32)
            nc.sync.dma_start(out=w2_sb[:], in_=w2_c.ap())
            lin2_sb = pp.tile([P, D2], f32)
            nc.sync.dma_start(out=lin2_sb[:], in_=lin2_c.ap())
            bias2_sb = pp.tile([P, D2], f32)
            nc.sync.dma_start(out=bias2_sb[:], in_=bias2_c.ap())
            idx_sb = pp.tile([P, nch], i32)
            nc.sync.dma_start(out=idx_sb[:], in_=idx_t[:, :])
            dstf_sb = pp.tile([P, nch], f32)
            nc.sync.dma_start(out=dstf_sb[:], in_=dstf_t[:, :])

            h1relu_sb = pp.tile([P, nt * P], f32)
            skip1_sb = pp.tile([P, nt * P], f32)
            sdst1_sb = pp.tile([P, nt], f32)
            sdst2_sb = pp.tile([P, nt], f32)

            ag1_in = dp.tile([nlocp, AGW], bf16)
            ag1_out = dp.tile([ncore * nlocp, AGW], bf16, addr_space="Shared")
            ag2_in = dp.tile([nlocp, AGW], bf16)
            ag2_out = dp.tile([ncore * nlocp, AGW], bf16, addr_space="Shared")

            # ================= Phase 1: GEMM1 ==============================
            for t in range(nt):
                xt_sb = wp.tile([P, k1, P], f32, tag="xt", bufs=3)
                nc.sync.dma_start(
                    out=xt_sb[:],
                    in_=xt_t[t * k1 * P:(t + 1) * k1 * P, :]
                        .rearrange("(k r) c -> r k c", k=k1))
                ps1 = pep.tile([P, c1], f32, tag="g1", space="PSUM")
                for k in range(k1):
                    nc.tensor.matmul(out=ps1[:], lhsT=xt_sb[:, k, :],
                                     rhs=w1_sb[:, k, :],
                                     start=(k == 0), stop=(k == k1 - 1))
                agt = wp.tile([P, AGW], bf16, tag="ag1t", bufs=3)
                nc.vector.tensor_copy(out=agt[:, 0:D1 + 1], in_=ps1[:, 0:D1 + 1])
                nc.vector.memset(agt[:, D1 + 1:D1 + 2], 1.0)
                nc.vector.memset(agt[:, D1 + 2:AGW], 0.0)
                nc.sync.dma_start(out=ag1_in[t * P:(t + 1) * P, :], in_=agt[:])
                nc.vector.tensor_copy(out=sdst1_sb[:, t:t + 1],
                                      in_=ps1[:, D1 + 1:D1 + 2])
                nc.scalar.copy(out=skip1_sb[:, t * P:(t + 1) * P],
                               in_=ps1[:, D1 + 2:c1])
            nc.gpsimd.collective_compute(
                "AllGather", mybir.AluOpType.bypass, replica_groups=rg,
                ins=[ag1_in[:].opt()], outs=[ag1_out[:].opt()])

            # ============ Edge phase (shared between both layers) ==========
            def edge_phase(ag_out, sdst_sb, evict_tile):
                for (t0, t1) in groups:
                    q0, q1 = int(chunk_t0[t0]), int(chunk_t0[t1])
                    gcn = q1 - q0
                    g_grp = gp.tile([P, gcn * AGW], bf16, tag="g")
                    esd = wp.tile([P, gcn], f32, tag="esd")
                    for t in range(t0, t1):
                        # per-tile broadcast of local s_dst values
                        tb = pep.tile([P, P], f32, tag="t", space="PSUM")
                        nc.tensor.transpose(
                            out=tb[:],
                            in_=sdst_sb[:, t:t + 1].to_broadcast([P, P]),
                            identity=ident_sb[:])
                        sdb = wp.tile([P, P], bf16, tag="sdb")
                        nc.scalar.copy(out=sdb[:], in_=tb[:])
                        for q in range(int(chunk_t0[t]), int(chunk_t0[t + 1])):
                            j = q - q0
                            nc.gpsimd.indirect_dma_start(
                                out=g_grp[:, j * AGW:(j + 1) * AGW],
                                out_offset=None,
                                in_=ag_out[:],
                                in_offset=bass.IndirectOffsetOnAxis(
                                    ap=idx_sb[:, q:q + 1], axis=0))
                            oh = mp.tile([P, P], bf16, tag="oh")
                            nc.vector.tensor_scalar(
                                out=oh[:], in0=iota_sb[:],
                                scalar1=dstf_sb[:, q:q + 1], scalar2=None,
                                op0=mybir.AluOpType.is_equal)
                            scr = mp.tile([P, P], bf16, tag="scr")
                            nc.vector.tensor_tensor(
                                out=scr[:], in0=oh[:], in1=sdb[:],
                                op=mybir.AluOpType.mult)
                            junk = mp.tile([P, P], bf16, tag="junk")
                            nc.scalar.activation(
                                out=junk[:], in_=scr[:],
                                func=mybir.ActivationFunctionType.Identity,
                                accum_out=esd[:, j:j + 1])
                    # batched scores for the group
                    u = wp.tile([P, gcn], f32, tag="u")
                    nc.vector.tensor_tensor(
                        out=u[:], in0=esd[:],
                        in1=g_grp[:].rearrange("p (k e) -> p k e", k=gcn)[:, :, D1],
                        op=mybir.AluOpType.add)
                    e1 = wp.tile([P, gcn], f32, tag="e1")
                    nc.scalar.activation(out=e1[:], in_=u[:],
                                         func=mybir.ActivationFunctionType.Exp)
                    e2 = wp.tile([P, gcn], f32, tag="e2")
                    nc.scalar.activation(out=e2[:], in_=u[:],
                                         func=mybir.ActivationFunctionType.Exp,
                                         scale=NEG)
                    pt = wp.tile([P, gcn], f32, tag="p")
                    nc.vector.tensor_tensor(out=pt[:], in0=e1[:], in1=e2[:],
                                            op=mybir.AluOpType.max)
                    # one-hot scale + matmul accumulate, per tile
                    for t in range(t0, t1):
                        pse = pep.tile([P, AGW], f32, tag="edge", space="PSUM")
                        qa, qb = int(chunk_t0[t]), int(chunk_t0[t + 1])
                        for q in range(qa, qb):
                            j = q - q0
                            oh2 = mp.tile([P, P], bf16, tag="oh2")
                            nc.vector.tensor_scalar(
                                out=oh2[:], in0=iota_sb[:],
                                scalar1=dstf_sb[:, q:q + 1],
                                scalar2=pt[:, j:j + 1],
                                op0=mybir.AluOpType.is_equal,
                                op1=mybir.AluOpType.mult)
                            nc.tensor.matmul(
                                out=pse[:], lhsT=oh2[:],
                                rhs=g_grp[:, j * AGW:(j + 1) * AGW],
                                start=(q == qa), stop=(q == qb - 1))
                        evict_tile(t, pse)

            # ---- layer-1 eviction: h1relu + GEMM2a + ag2 assembly ---------
            def evict1(t, pse):
                den = wp.tile([P, 1], f32, tag="den")
                nc.vector.tensor_scalar(out=den[:], in0=pse[:, D1 + 1:D1 + 2],
                                        scalar1=EPS, scalar2=None,
                                        op0=mybir.AluOpType.add)
                rec = wp.tile([P, 1], f32, tag="rec")
                nc.vector.reciprocal(out=rec[:], in_=den[:])
                tmp = wp.tile([P, P], f32, tag="ev1a")
                nc.vector.tensor_scalar(out=tmp[:], in0=pse[:, 0:D1],
                                        scalar1=rec[:, 0:1], scalar2=None,
                                        op0=mybir.AluOpType.mult)
                tmp2 = wp.tile([P, P], f32, tag="ev1b")
                nc.vector.tensor_tensor(out=tmp2[:], in0=tmp[:],
                                        in1=skip1_sb[:, t * P:(t + 1) * P],
                                        op=mybir.AluOpType.add)
                nc.scalar.activation(out=h1relu_sb[:, t * P:(t + 1) * P],
                                     in_=tmp2[:],
                                     func=mybir.ActivationFunctionType.Relu)
                tp = pep.tile([P, P], f32, tag="t", space="PSUM")
                nc.tensor.transpose(out=tp[:],
                                    in_=h1relu_sb[:, t * P:(t + 1) * P],
                                    identity=ident_sb[:])
                h1T = wp.tile([P, P], f32, tag="h1T")
                nc.scalar.copy(out=h1T[:], in_=tp[:])
                sc = pep.tile([P, 2], f32, tag="g1", space="PSUM")
                nc.tensor.matmul(out=sc[:], lhsT=h1T[:], rhs=w2a_sb[:],
                                 start=True, stop=True)
                agt = wp.tile([P, AGW], bf16, tag="ag2t", bufs=3)
                nc.vector.tensor_copy(out=agt[:, 0:D1],
                                      in_=h1relu_sb[:, t * P:(t + 1) * P])
                nc.vector.tensor_copy(out=agt[:, D1:D1 + 1], in_=sc[:, 0:1])
                nc.vector.memset(agt[:, D1 + 1:D1 + 2], 1.0)
                nc.vector.memset(agt[:, D1 + 2:AGW], 0.0)
                nc.sync.dma_start(out=ag2_in[t * P:(t + 1) * P, :], in_=agt[:])
                nc.vector.tensor_copy(out=sdst2_sb[:, t:t + 1], in_=sc[:, 1:2])

            edge_phase(ag1_out, sdst1_sb, evict1)
            nc.gpsimd.collective_compute(
                "AllGather", mybir.AluOpType.bypass, replica_groups=rg,
                ins=[ag2_in[:].opt()], outs=[ag2_out[:].opt()])

            # ---- layer-2 eviction: out = (agg2/den)@W2 + h1relu@lin2 + b --
            def evict2(t, pse):
                den = wp.tile([P, 1], f32, tag="den")
                nc.vector.tensor_scalar(out=den[:], in0=pse[:, D1 + 1:D1 + 2],
                                        scalar1=EPS, scalar2=None,
                                        op0=mybir.AluOpType.add)
                rec = wp.tile([P, 1], f32, tag="rec")
                nc.vector.reciprocal(out=rec[:], in_=den[:])
                a2n = wp.tile([P, P], f32, tag="a2n")
                nc.vector.tensor_scalar(out=a2n[:], in0=pse[:, 0:D1],
                                        scalar1=rec[:, 0:1], scalar2=None,
                                        op0=mybir.AluOpType.mult)
                tp1 = pep.tile([P, P], f32, tag="t", space="PSUM")
                nc.tensor.transpose(out=tp1[:], in_=a2n[:], identity=ident_sb[:])
                a2T = wp.tile([P, P], f32, tag="a2T")
                nc.scalar.copy(out=a2T[:], in_=tp1[:])
                tp2 = pep.tile([P, P], f32, tag="t", space="PSUM")
                nc.tensor.transpose(out=tp2[:],
                                    in_=h1relu_sb[:, t * P:(t + 1) * P],
                                    identity=ident_sb[:])
                h1T = wp.tile([P, P], f32, tag="h1T")
                nc.scalar.copy(out=h1T[:], in_=tp2[:])
                po = pep.tile([P, D2], f32, tag="out", space="PSUM")
                nc.tensor.matmul(out=po[:], lhsT=a2T[:], rhs=w2_sb[:],
                                 start=True, stop=False)
                nc.tensor.matmul(out=po[:], lhsT=h1T[:], rhs=lin2_sb[:],
                                 start=False, stop=True)
                osb = wp.tile([P, D2], f32, tag="osb", bufs=3)
                nc.vector.tensor_tensor(out=osb[:], in0=po[:], in1=bias2_sb[:],
                                        op=mybir.AluOpType.add)
                rows = min(nloc - t * P, P)
                nc.sync.dma_start(out=out_t[t * P:t * P + rows, :],
                                  in_=osb[:rows, :])

            edge_phase(ag2_out, sdst2_sb, evict2)

    nc.compile()
    return nc


def _run(nc, in_maps, ncore, trace=False, **kw):
    from concourse import bass_utils
    return bass_utils.run_bass_kernel_spmd(
        nc, in_maps, core_ids=list(range(ncore)), trace=trace, **kw)


_CACHE = {}


def kernel(**inputs):
    x = np.asarray(inputs['x'], np.float32)
    edge_index = np.asarray(inputs['edge_index'], np.int32)
    cfg = _cfg(N=100000, E=400000, DIN=300, D1=128, D2=512, ncore=8)
    weights = {k: np.asarray(v, np.float32) for k, v in inputs.items()
               if k not in ('x', 'edge_index')}

    aux, cpt_t = preprocess(x, edge_index, cfg)
    key = ('nn_gat', int(cpt_t.sum()))
    if key not in _CACHE:
        _CACHE[key] = build(cfg, cpt_t, weights)
    nc = _CACHE[key]

    in_maps = []
    for c in range(cfg['NCORE']):
        in_maps.append(dict(
            xt_tiles=make_xt_tiles(x, cfg, c),
            src_gidx=aux[c]['src_gidx'],
            dstloc_f=aux[c]['dstloc_f'],
        ))
    res = _run(nc, in_maps, cfg['NCORE'])
    out = np.concatenate([res.results[c]['out'] for c in range(cfg['NCORE'])],
                         axis=0)
    return out
